# revision 28
# baseline (speedup 1.0000x reference)
"""Trainium2 Bass kernel for the CrossAttention (linear-attention style) module.

Math (per batch b, head h, stream s in {rgb, x}):
    K = A_s @ Wk_s^T, V = A_s @ Wv_s^T            (A_s = stream input [N, C])
    ctx_s = softmax(scale * K^T V, axis=rows)     # [32, 32] per head
    out_s = A_s @ blockdiag(ctx_{s'})             # s' = the OTHER stream

Key identity: K^T V = Wk (A^T A) Wv^T, so the big inputs only feed the Gram
matrix G = A^T A (one [256,256] per (batch, stream)); the rest is tiny.

Sharding (v2, collective-free): 8 cores = 4 batches x 2 output streams.
Core 2b+s OUTPUTS stream s of batch b.  It loads TWO things:
  - a_pm: the OTHER stream A_{s'} row-major (partition-major tiles) -> Gram
    G = A_{s'}^T A_{s'} -> ctx_{s'} computed entirely locally, and
  - aT:   its own stream A_s already TRANSPOSED BY THE HOST (c-major), the
    moving operand for out_s^T = blockdiag(ctx_{s'}) @ A_s^T.
This removes the AllReduce (measured ~25 us of pure latency on the critical
path: 11.6 us trigger-start delay + 13.2 us transfer for 32 KB) and the
256 PE transposes + their ~50 us of PSUM->SBUF copies.  The price is a
second 8 MB input stream per core, which rides in the ~47 us window where
the baseline's DMA engines sat idle.  Everything is now a single DMA-paced
pipeline: stream a_pm -> Gram chases it -> ctx (~4 us of tiny matmuls +
softmax) -> 64 block-stationary out matmuls chase the aT stream -> fp16
staging copies (DVE/ACT alternate; PSUM-source copies are 1x mode,
~676 ns per [128,512]) -> output DMA (sync HWDGE + gpsimd SWDGE alternate).

Hardware notes baked in:
 - Each Gram accumulation region gets its OWN PSUM bank: a start=True
   matmul clears has_written BANK-WIDE, so interleaving two accumulation
   groups in one bank corrupts the other region's in-flight tile.
 - fp16 everywhere on the streamed paths: PE full rate, DMA traffic
   halved.  fp8 for the Gram was analyzed and rejected: logits have
   spread ~±180 with frequent near-ties, and softmax amplifies the ~4%
   fp8 Gram error far past the 2e-2 gate (fp16 already amplifies its
   0.03% quantization to 5.6e-3 measured).
"""

import sys

if "/opt/trn_rl_repo" not in sys.path:
    sys.path.insert(0, "/opt/trn_rl_repo")

import numpy as np

import concourse.bass as bass
import concourse.mybir as mybir
import concourse.tile as tile
from concourse import bacc
from concourse.bass import ds, ts
from concourse.bass_utils import run_bass_kernel_spmd

P = 128
C = 256
HD = 32
SCALE = HD ** -0.5
F16 = mybir.dt.float16
F32 = mybir.dt.float32

B_FULL = 4
N_FULL = 16384
H_FULL = 8

N_TILES = N_FULL // P          # 128
TPC = 16                       # tiles per chunk
N_CHUNKS = N_TILES // TPC      # 8
SPAN = 4                       # aT tiles per out-matmul -> N = 512


def build_module(num_devices=8):
    nc = bacc.Bacc(
        "TRN2",
        target_bir_lowering=False,
        debug=False,
        enable_asserts=False,
        num_devices=num_devices,
    )
    a_pm = nc.dram_tensor("a_pm", [P, N_TILES, C], F16, kind="ExternalInput").ap()
    aT_d = nc.dram_tensor("aT", [P, 2, N_FULL], F16, kind="ExternalInput").ap()
    wkT = nc.dram_tensor("wkT", [P, 2, C], F16, kind="ExternalInput").ap()
    wvT = nc.dram_tensor("wvT", [P, 2, C], F16, kind="ExternalInput").ap()
    ident_d = nc.dram_tensor("ident", [P, P], F16, kind="ExternalInput").ap()
    oT = nc.dram_tensor("oT", [P, 2, N_FULL], F16, kind="ExternalOutput").ap()

    with tile.TileContext(nc) as tc:
        _build_kernel(tc, a_pm, aT_d, wkT, wvT, ident_d, oT)
    nc.compile()
    return nc


def _build_kernel(tc, a_pm, aT_d, wkT_d, wvT_d, ident_d, oT):
    nc = tc.nc

    with (
        tc.tile_pool(name="persist", bufs=1) as persist,
        tc.tile_pool(name="stage", bufs=3) as stage,
        tc.tile_pool(name="psum_ga", bufs=1, space="PSUM") as psum_ga,
        tc.tile_pool(name="psum_gb", bufs=1, space="PSUM") as psum_gb,
        tc.tile_pool(name="psum_big", bufs=3, space="PSUM") as psum_big,
    ):
        # ---- persistent SBUF state ----
        in_sb = [
            persist.tile([P, TPC, C], F16, tag=f"in{ch}", name=f"in{ch}")
            for ch in range(N_CHUNKS)
        ]
        aT = persist.tile([P, 2, N_FULL], F16, tag="aT", name="aT")
        w_k = persist.tile([P, 2, C], F16, tag="w_k")
        w_v = persist.tile([P, 2, C], F16, tag="w_v")
        ident = persist.tile([P, P], F16, tag="ident")
        g16 = persist.tile([P, 2, C], F16, tag="g16")
        T16 = persist.tile([P, 2, C], F16, tag="T16")
        lgc = persist.tile([P, 2, HD], F32, tag="lgc")    # compact logits^T
        cT_own = persist.tile([P, 2, HD], F32, tag="cT_own")
        ctx16 = persist.tile([P, 2, P], F16, tag="ctx16")  # blockdiag, fp16

        # ---- input streams.  a_pm (gates Gram -> ctx -> everything) owns the
        # sync HWDGE ring at the start; aT rides the SECOND HWDGE ring
        # (scalar / qActDynamicHW) but is dep-chained behind a_pm's last DMA
        # so it doesn't steal engine time from the critical stream.  The out
        # DMAs later reuse the sync ring, which by then is drained -- the
        # gpsimd SWDGE ring (measured 10+ us trigger->completion lag that
        # convoyed the whole out pipeline) only carries the tiny weights.
        # chunk 0 in pieces of [2, 2, 4, 8] tiles so the first Gram matmul
        # can start as early as possible on a small first transfer.
        t0 = 0
        for npc in (1, 1, 2, 4, 8):
            nc.sync.dma_start(
                in_sb[0][:, ds(t0, npc), :], a_pm[:, ds(t0, npc), :]
            )
            t0 += npc
        for ch in range(1, N_CHUNKS):
            nc.sync.dma_start(in_sb[ch][:], a_pm[:, ts(ch, TPC), :])
        # weights between a_pm and aT on the same ring: needed at ~40 us,
        # land at ~28.  NO SWDGE DMAs anywhere: SDMA engine 15 is ~18%
        # slower when the SWDGE descriptor rings are in play (they live on
        # SBUF partitions whose AXI ports serve engines 7/15) and as the
        # fully-busy straggler it sets the kernel's critical path.
        nc.sync.dma_start(w_k[:], wkT_d)
        nc.sync.dma_start(w_v[:], wvT_d)
        nc.sync.dma_start(ident[:], ident_d)
        # aT follows a_pm on the SAME sync HWDGE ring: FIFO within the ring
        # is the only reliable priority mechanism (SWDGE bulk transfers
        # measured ~150 GB/s and an instruction-level dep only sequences the
        # trigger, not the data).  8 pieces in exact consumption order.
        for g in range(2):
            for quarter in range(4):
                nc.sync.dma_start(
                    aT[:, g, ts(quarter, N_FULL // 4)],
                    aT_d[:, g, ts(quarter, N_FULL // 4)],
                )
        nc.vector.memset(ctx16[:], 0.0)

        # ---- HAM warm-up: ~20 throwaway matmuls on the zeroed ctx16 tile
        # keep the PE busy from ~7 us so the SHORT window fires and the real
        # Gram runs at 2.4 GHz instead of spending its first ~4 us at 1.2.
        for w in range(20):
            pw = psum_big.tile([P, 2, SPAN * P], F32, tag="big", name=f"warm{w}")
            nc.tensor.matmul(
                pw[:, 0, ts(0, P)], ctx16[:, 0, :], ctx16[:, 0, :],
                start=True, stop=True,
            )

        # ---- phase 1: Gram G = A^T A; one accumulation region PER BANK ----
        # Triangle form (G symmetric): [G11|G12] (256 moving cols) and G22
        # (128 cols) -- the Gram is the head of the critical path and is
        # MM-column-bound at ~2.4 GHz, so the 25% column cut is real time.
        # G21 = G12^T reconstructed below with one PE transpose.
        pga = psum_ga.tile([P, 2, C], F32, tag="ga", name="pga")  # G[0:128, :]
        pgb = psum_gb.tile([P, 2, C], F32, tag="gb", name="pgb")  # G22 in [:128]
        for ch in range(N_CHUNKS):
            for t in range(TPC):
                ti = ch * TPC + t
                tile_ap = in_sb[ch][:, t, :]
                nc.tensor.matmul(
                    pga[:, 0, :], tile_ap[:, ts(0, P)], tile_ap,
                    start=(ti == 0), stop=(ti == N_TILES - 1),
                )
                nc.tensor.matmul(
                    pgb[:, 0, ts(0, P)], tile_ap[:, ts(1, P)], tile_ap[:, ts(1, P)],
                    start=(ti == 0), stop=(ti == N_TILES - 1),
                )
        nc.vector.tensor_copy(g16[:, 0, ts(1, P)], pga[:, 0, ts(1, P)])
        nc.scalar.copy(g16[:, 0, ts(0, P)], pga[:, 0, ts(0, P)])
        nc.scalar.copy(g16[:, 1, ts(1, P)], pgb[:, 0, ts(0, P)])
        # G21 = G12^T: PE transpose of the fp16 G12 block via identity matmul
        pt21 = psum_big.tile([P, 2, SPAN * P], F32, tag="big", name="pt21")
        nc.tensor.matmul(
            pt21[:, 0, ts(0, P)], g16[:, 0, ts(1, P)], ident[:],
            start=True, stop=True,
        )
        nc.vector.tensor_copy(g16[:, 1, ts(0, P)], pt21[:, 0, ts(0, P)])

        # ---- ctx: T = G @ Wk^T (G symmetric), logits = Wv @ T ----
        # pT blocks reuse the two Gram banks (same tag ring -> sequenced).
        pTA = psum_ga.tile([P, 2, C], F32, tag="ga", name="pTA")  # T[0:128, :]
        pTB = psum_gb.tile([P, 2, C], F32, tag="gb", name="pTB")  # T[128:256, :]
        for blkc, pT in ((0, pTA), (1, pTB)):
            for ci in range(2):
                nc.tensor.matmul(
                    pT[:, 0, :], g16[:, ci, ts(blkc, P)], w_k[:, ci, :],
                    start=(ci == 0), stop=(ci == 1),
                )
        nc.vector.tensor_copy(T16[:, 0, ts(0, P)], pTA[:, 0, ts(0, P)])
        nc.vector.tensor_copy(T16[:, 1, ts(0, P)], pTB[:, 0, ts(0, P)])
        nc.scalar.copy(T16[:, 0, ts(1, P)], pTA[:, 0, ts(1, P)])
        nc.scalar.copy(T16[:, 1, ts(1, P)], pTB[:, 0, ts(1, P)])

        for g in range(2):
            pl2 = psum_big.tile([P, 2, SPAN * P], F32, tag="big", name=f"pl{g}")
            pl = pl2[:, 0, ts(0, P)]
            for ci in range(2):
                nc.tensor.matmul(
                    pl, w_v[:, ci, ts(g, P)], T16[:, ci, ts(g, P)],
                    start=(ci == 0), stop=(ci == 1),
                )
            # extract the 4 diagonal head blocks -> compact [128, 32]
            for h in range(4):
                eng = nc.vector if h % 2 == 0 else nc.scalar
                if eng is nc.vector:
                    eng.tensor_copy(
                        lgc[ds(h * HD, HD), g, :],
                        pl2[ds(h * HD, HD), 0, ds(h * HD, HD)],
                    )
                else:
                    eng.copy(
                        lgc[ds(h * HD, HD), g, :],
                        pl2[ds(h * HD, HD), 0, ds(h * HD, HD)],
                    )
            # batched softmax over d (free axis) for all 4 heads at once
            mx = stage.tile([P, 1], F32, tag="mx", name=f"mx{g}")
            nc.vector.tensor_reduce(
                mx[:], lgc[:, g, :], axis=mybir.AxisListType.X, op=mybir.AluOpType.max
            )
            nmx = stage.tile([P, 1], F32, tag="nmx", name=f"nmx{g}")
            nc.vector.tensor_scalar_mul(nmx[:], mx[:], -SCALE)
            sm = stage.tile([P, HD], F32, tag="sm", name=f"sm{g}")
            ssum = stage.tile([P, 1], F32, tag="ssum", name=f"ssum{g}")
            nc.scalar.activation(
                sm[:], lgc[:, g, :], mybir.ActivationFunctionType.Exp,
                bias=nmx[:], scale=SCALE, accum_out=ssum[:],
            )
            rs = stage.tile([P, 1], F32, tag="rs", name=f"rs{g}")
            nc.vector.reciprocal(rs[:], ssum[:])
            smn = stage.tile([P, HD], F32, tag="smn", name=f"smn{g}")
            nc.vector.tensor_scalar_mul(smn[:], sm[:], rs[:])
            # per-head 32x32 transpose: [32h+e, d] -> [32h+d, e]
            nc.vector.transpose(cT_own[:, g, :], smn[:])
            # blockdiag fill for THIS g immediately (so g=0 out matmuls do
            # not queue behind g=1's softmax on the DVE FIFO); DVE/gpsimd.
            for h in range(4):
                dst = ctx16[ds(h * HD, HD), g, ds(h * HD, HD)]
                srcc = cT_own[ds(h * HD, HD), g, :]
                eng = nc.vector if h % 2 == 0 else nc.gpsimd
                eng.tensor_copy(dst, srcc)

        # ---- out^T = ctx_blk (stationary) @ aT spans, fp16 staged, DMA out.
        # The two copy engines alternate per 512-col matmul; DMAs ship GROUPS
        # of GRP matmuls (GRP*SPAN*P cols = 4 KB per partition) so the HBM
        # write descriptors are big enough for full DMA rate (1 KB
        # descriptors measured only ~270 GB/s; the small-descriptor penalty
        # is HBM-side).  Groups alternate sync HWDGE / gpsimd SWDGE rings.
        # Copies move PAIRS of spans ([128, 1024] across 2 consecutive PSUM
        # banks) per instruction: (1024+151)/0.96 GHz = 1.22 us vs 2 x 0.69,
        # so the two copy engines supply ~420 GB/s -- matching the SDMA
        # engines exactly.
        GRP = 8
        QG = N_TILES // SPAN // GRP                      # 4 groups per g
        for g in range(2):
            for qg in range(QG):
                last_group = g == 1 and qg == QG - 1
                if not last_group:
                    stg = stage.tile(
                        [P, GRP * SPAN * P], F16, tag="st", bufs=4,
                        name=f"st{g}_{qg}"
                    )
                for jp in range(GRP // 2):
                    po2 = psum_big.tile(
                        [P, 2, SPAN * P], F32, tag="big", name=f"po{g}_{qg}_{jp}"
                    )
                    for h in range(2):
                        q = qg * GRP + jp * 2 + h
                        nc.tensor.matmul(
                            po2[:, h, :], ctx16[:, g, :], aT[:, g, ts(q, SPAN * P)],
                            start=True, stop=True,
                        )
                    if last_group:
                        # taper: ship each pair as its own small DMA so the
                        # final data chunk after the final copy is 256 KB,
                        # not 1 MB -- trims ~2 us off the kernel tail.
                        st2 = stage.tile(
                            [P, 2 * SPAN * P], F16, tag="st2", bufs=4,
                            name=f"st2_{jp}"
                        )
                        if jp % 2 == 0:
                            nc.vector.tensor_copy(st2[:], po2[:, :, :])
                        else:
                            nc.scalar.copy(st2[:], po2[:, :, :])
                        nc.sync.dma_start(
                            oT[:, g, ds((qg * GRP + jp * 2) * SPAN * P,
                                        2 * SPAN * P)],
                            st2[:],
                        )
                    else:
                        dst = stg[:, ds(jp * 2 * SPAN * P, 2 * SPAN * P)]
                        if jp % 2 == 0:
                            nc.vector.tensor_copy(dst, po2[:, :, :])
                        else:
                            nc.scalar.copy(dst, po2[:, :, :])
                if not last_group:
                    nc.sync.dma_start(
                        oT[:, g, ts(qg, GRP * SPAN * P)], stg[:]
                    )


# ---------------------------------------------------------------------------
# Host-side wrapper
# ---------------------------------------------------------------------------

_NC_CACHE = {}


def _get_module(**kw):
    key = tuple(sorted(kw.items()))
    if key not in _NC_CACHE:
        _NC_CACHE[key] = build_module(**kw)
    return _NC_CACHE[key]


def make_in_maps(rgb, x, Wkv_rgb, Wkv_x, n_cores=8):
    """Per-core inputs. Core 2b+s OUTPUTS stream s (0=rgb, 1=x) of batch b:
    it grams the OTHER stream (whose ctx it needs) and streams its own
    stream host-transposed for the out matmuls."""
    in_maps = []
    for core in range(n_cores):
        b, s = divmod(core, 2)
        A_out = (rgb if s == 0 else x)[b]          # stream we output
        A_gram = (x if s == 0 else rgb)[b]         # stream whose ctx we need
        W = Wkv_x if s == 0 else Wkv_rgb           # weights of the gram stream
        ag16 = A_gram.astype(np.float16)
        a_pm = np.ascontiguousarray(ag16.reshape(N_TILES, P, C).transpose(1, 0, 2))
        ao16 = A_out.astype(np.float16)
        aT = np.ascontiguousarray(ao16.T.reshape(2, P, N_FULL).transpose(1, 0, 2))
        WkT = W[:C].T.reshape(2, P, C).transpose(1, 0, 2)   # [p, ci, col]
        WvT = W[C:].T.reshape(2, P, C).transpose(1, 0, 2)
        in_maps.append(
            {
                "a_pm": a_pm,
                "aT": aT,
                "wkT": np.ascontiguousarray(WkT.astype(np.float16)),
                "wvT": np.ascontiguousarray(WvT.astype(np.float16)),
                "ident": np.eye(P, dtype=np.float16),
            }
        )
    return in_maps


def assemble(results):
    out_rgb = np.empty((B_FULL, N_FULL, C), dtype=np.float32)
    out_x = np.empty_like(out_rgb)
    for core, res in enumerate(results):
        b, s = divmod(core, 2)
        o = res["oT"].transpose(2, 1, 0).reshape(N_FULL, C).astype(np.float32)
        (out_rgb if s == 0 else out_x)[b] = o
    return out_rgb, out_x


def kernel(rgb, x, Wkv_rgb, Wkv_x, num_heads):
    rgb = np.asarray(rgb, dtype=np.float32)
    x = np.asarray(x, dtype=np.float32)
    Wkv_rgb = np.asarray(Wkv_rgb, dtype=np.float32)
    Wkv_x = np.asarray(Wkv_x, dtype=np.float32)
    assert int(num_heads) == H_FULL
    assert rgb.shape == (B_FULL, N_FULL, C) and x.shape == (B_FULL, N_FULL, C)

    nc = _get_module()
    in_maps = make_in_maps(rgb, x, Wkv_rgb, Wkv_x)
    res = run_bass_kernel_spmd(nc, in_maps, core_ids=list(range(8)))
    return assemble(res.results)
